# revision 23
# baseline (speedup 1.0000x reference)
"""Trainium2 Bass kernel for a BiQRNN3D layer.

reference math:
  gates = conv3d(x, W, SAME, 3x3x3) + b          x: [2,16,31,256,256] f32
  Z, F1, F2 = split(gates, 3, channel)           W: [48,16,3,3,3], b: [48]
  Z = tanh(Z); F1 = sigmoid(F1); F2 = sigmoid(F2)
  h_fwd: depth-forward  recurrence h = F1*h + (1-F1)*Z
  h_bwd: depth-backward recurrence h = F2*h + (1-F2)*Z
  out = h_fwd + h_bwd                            [2,16,31,256,256] f32

Distribution: H (=256) is sharded 32 rows per core across 8 NeuronCores
(SPMD, identical program; each core's x shard carries its 1-row conv halo
with global-edge zeros baked in by the host).

Per-core pipeline (v2 - no DRAM gates round-trip):
  * conv as matmul, K = (kd,ci) = 48 contraction rows. The moving x tile
    holds 3 kd-shifted copies in partitions 0-47 (block A) and an
    additional h+1-shifted copy in partitions 64-111 (block B,
    host-prepared). Partition 48 is a ones-row (bias rides as a stationary
    row); partitions 49-63 are zeros.
  * M = 96: stationary columns (j, co) produce BOTH output h rows of an
    h-block at once. Per psum tile [96, 2*256] six K=112 matmuls
    accumulate: passes (p in {0,1}) x (kw in {0,1,2}); pass p streams x
    rows at tile-h 2p, and blocks A/B provide taps kh = 2p-j and 2p+1-j.
  * gates evac psum -> SBUF f16 tile g[96, 32, 256] (d-major). Slot 31
    holds a "j-swap" copy of d=30 for j=1 (6 extra M=48 matmuls with
    j-swapped stationary columns), so both d=30 transposes below read
    from partition base 0.
  * on-chip transpose via PE is_transpose matmuls into PSUM f16 tile
    tp[128, 3 banks, 1024]: per w-half, 30x [96,128] transposes at slot
    (d//10, (d%10)*96) plus 2x [48,128] for d=30 into the 128-f16 bank
    gaps (offset 960). No matmul group crosses a 2KB psum bank.
  * ACT: tanh/sigmoid read strided from psum tp at 128-partition
    utilization; DVE: g = (f-1)*z, tensor_tensor_scan (h = f*h - g) for
    both directions (backward stored d-reversed); f zeroed at d=0 so one
    long scan chains safely across channel runs. out fp32 [S, 16, 31]
    -> host reassembles.
"""

from contextlib import ExitStack

import numpy as np

import concourse.bass as bass
import concourse.tile as tile
from concourse import bacc, mybir

F32 = mybir.dt.float32
F16 = mybir.dt.float16
AF = mybir.ActivationFunctionType
ALU = mybir.AluOpType

N_CORES = 8
B = 2
CIN = 16
HID = 16
CO = 3 * HID            # 48
D = 31
H = 256
W = 256
HSH = H // N_CORES      # 32
HB = 2                  # output h rows per conv tile (= M/CO)
DC = 2                  # d slices per psum tile
WP = W + 2
S = B * HSH * W         # 16384
FX = D * 2 * WP         # x tile free extent per partition
CHUNK = 128
NST = 6                 # stationary matrices (main)
WCOLS = NST * 2 * CO + NST * CO   # 576 + 288 = 864


def _build_program(reps=1, do_conv=True, do_scan=True, do_tp=True):
    nc = bacc.Bacc("TRN2", target_bir_lowering=False, debug=False)

    x_dram = nc.dram_tensor("x", [CIN, D + 2, B, HSH + 2, WP], F16,
                            kind="ExternalInput").ap()
    wts = nc.dram_tensor("wts", [128, WCOLS], F16,
                         kind="ExternalInput").ap()
    aux = nc.dram_tensor("aux", [16, FX], F16, kind="ExternalInput").ap()
    idn = nc.dram_tensor("idn", [96, 96], F16, kind="ExternalInput").ap()
    out = nc.dram_tensor("out", [S, HID, D], F32, kind="ExternalOutput").ap()

    with tile.TileContext(nc) as tc, ExitStack() as ctx:
        wsb = nc.alloc_sbuf_tensor("wsb", [128, WCOLS], F16).ap()
        idsb = nc.alloc_sbuf_tensor("idsb", [96, 96], F16).ap()
        # x tile: A rows hold x at h = h0 + 2t, B rows x at h0 + 1 + 2t
        NXB = 3
        xbufs = [nc.alloc_sbuf_tensor(f"xb{i}", [112, D, 2, WP], F16).ap()
                 for i in range(NXB)]

        nc.sync.dma_start(wsb, wts)
        nc.sync.dma_start(idsb, idn)
        for xb in xbufs:
            nc.sync.dma_start(
                xb[48:64].rearrange("p a b c -> p (a b c)"), aux)

        g_pool = ctx.enter_context(tc.tile_pool(name="gp", bufs=2))
        ps_pool = ctx.enter_context(tc.tile_pool(name="ps", bufs=2,
                                                 space="PSUM"))
        tp_pool = ctx.enter_context(tc.tile_pool(name="tp", bufs=2,
                                                 space="PSUM"))
        sc_pool = ctx.enter_context(tc.tile_pool(name="sc", bufs=10))
        o_pool = ctx.enter_context(tc.tile_pool(name="op", bufs=3))

        n_hblk = HSH // HB
        n_dc = (D + DC - 1) // DC

        def scan_pieces(tp, j, c0):
            """One 128-pixel chunk. Activations are emitted immediately
            (they read the psum tp tile); the post-act SBUF work is
            returned as deferred pieces, drained two h-blocks later so
            its deps are long-satisfied and it fills engine idle time
            without head-of-line blocking the conv-critical evacs."""
            zt = sc_pool.tile([128, HID, D], F16, tag="zt", name="zt")
            f1 = sc_pool.tile([128, HID, D], F16, tag="f1", name="f1")
            f2 = sc_pool.tile([128, HID, D], F16, tag="f2", name="f2")

            def act_gate(gi, dst, fn):
                # main d 0..29 at psum slot (d//10, (d%10)*96), cols j*48+c
                vm = tp[:, :, 0:960].rearrange(
                    "p b (db c) -> p c b db", c=96)
                cb = j * CO + gi * HID
                nc.scalar.activation(
                    dst[:, :, 0:30].rearrange("p c (b db) -> p c b db", b=3),
                    vm[:, cb:cb + HID], fn)
                # d=30 lives in the bank-j gap at offset 960, cols 0-47
                nc.scalar.activation(
                    dst[:, :, 30:31],
                    tp[:, j:j + 1, 960 + gi * HID:960 + (gi + 1) * HID]
                    .rearrange("p a c -> p c a"), fn)

            act_gate(0, zt, AF.Tanh)
            act_gate(1, f1, AF.Sigmoid)
            act_gate(2, f2[:, :, ::-1], AF.Sigmoid)

            st = {}

            def order_tok(f, gcur, gd0):
                # 1-element self-bypass whose second operand reads the
                # just-evacuated g region: pins this piece AFTER that evac
                # in the scheduler's dependency-topological order.
                if gcur is not None:
                    nc.vector.tensor_tensor(
                        f[0:1, 0:1, 0:1], f[0:1, 0:1, 0:1],
                        gcur[0:1, gd0:gd0 + 1, 0:1], ALU.bypass)

            def p1(gcur=None, gd0=0):
                order_tok(f1, gcur, gd0)
                st['g1'] = g1 = sc_pool.tile([128, HID, D], F16, tag="g1",
                                             name="g1")
                nc.vector.scalar_tensor_tensor(
                    g1[:], f1[:], 1.0, zt[:], ALU.subtract, ALU.mult)
                nc.vector.memset(f1[:, :, 0:1], 0.0)
                st['h1'] = h1 = sc_pool.tile([128, HID, D], F16, tag="h1",
                                             name="h1")
                nc.vector.tensor_tensor_scan(
                    h1[:].rearrange("p c d -> p (c d)"),
                    f1[:].rearrange("p c d -> p (c d)"),
                    g1[:].rearrange("p c d -> p (c d)"),
                    0.0, ALU.mult, ALU.subtract)

            def p2(gcur=None, gd0=0):
                order_tok(f2, gcur, gd0)
                h1 = st['h1']
                g2 = sc_pool.tile([128, HID, D], F16, tag="g2", name="g2")
                nc.vector.scalar_tensor_tensor(
                    g2[:], f2[:], 1.0, zt[:, :, ::-1],
                    ALU.subtract, ALU.mult)
                nc.vector.memset(f2[:, :, 0:1], 0.0)
                h2 = sc_pool.tile([128, HID, D], F16, tag="h2", name="h2")
                nc.vector.tensor_tensor_scan(
                    h2[:].rearrange("p c d -> p (c d)"),
                    f2[:].rearrange("p c d -> p (c d)"),
                    g2[:].rearrange("p c d -> p (c d)"),
                    0.0, ALU.mult, ALU.subtract)
                o = o_pool.tile([128, HID, D], F32, tag="o", name="o")
                nc.gpsimd.tensor_add(o[:], h1[:], h2[:, :, ::-1])
                nc.gpsimd.dma_start(out[c0:c0 + CHUNK], o[:])

            return [p1, p2]

        tix = 0
        pending = []   # drained two h-blocks later
        fresh = []
        for _rep in range(reps):
            for b_i in range(B):
                for hb_i in range(n_hblk):
                    xb = xbufs[tix % NXB]
                    tix += 1
                    h0 = hb_i * HB
                    dslices = ([(0, 6), (6, 31)] if tix == 1 else
                               [(0, 31)])
                    for kd in range(3):
                        for ds0, ds1 in dslices:
                            nc.sync.dma_start(
                                xb[kd * 16:kd * 16 + 16, ds0:ds1].rearrange(
                                    "p d t w -> p d (t w)"),
                                x_dram[:, kd + ds0:kd + ds1, b_i,
                                       h0:h0 + 2, :].rearrange(
                                    "p d t w -> p d (t w)"))
                            nc.sync.dma_start(
                                xb[64 + kd * 16:64 + kd * 16 + 16,
                                   ds0:ds1].rearrange(
                                    "p d t w -> p d (t w)"),
                                x_dram[:, kd + ds0:kd + ds1, b_i,
                                       h0 + 2:h0 + 4, :].rearrange(
                                    "p d t w -> p d (t w)"))
                    s0 = b_i * (HSH * W) + h0 * W
                    g = g_pool.tile([96, 32, W], F16, tag="g")
                    for dc in range(n_dc if do_conv else 0):
                        d0 = dc * DC
                        dn = min(DC, D - d0)
                        ps = ps_pool.tile([96, DC * W], F32, tag="ps")
                        psv = ps[:, 0:dn * W].rearrange(
                            "p (d w) -> p d w", w=W)
                        k = 0
                        for p in range(2):
                            for kw in range(3):
                                nc.tensor.matmul(
                                    psv,
                                    wsb[0:112, k * 96:(k + 1) * 96],
                                    xb[0:112, d0:d0 + dn, p, kw:kw + W],
                                    start=(k == 0), stop=(k == NST - 1))
                                k += 1
                        if dc == n_dc - 1:
                            # j-swap extra: d=30 gates for j=1 at rows 0-47
                            k = 0
                            for p in range(2):
                                for kw in range(3):
                                    nc.tensor.matmul(
                                        ps[0:48, W:2 * W],
                                        wsb[0:112,
                                            576 + k * 48:576 + (k + 1) * 48],
                                        xb[0:112, d0, p, kw:kw + W],
                                        start=(k == 0), stop=(k == NST - 1))
                                    k += 1
                        gv = g[:, d0:d0 + dn, :].rearrange("p d w -> p (d w)")
                        nc.vector.tensor_copy(gv, ps[:, 0:dn * W])
                        if dc == n_dc - 1:
                            nc.vector.tensor_copy(
                                g[0:48, 31, :], ps[0:48, W:2 * W])
                        last_blk = (_rep == reps - 1 and b_i == B - 1
                                    and hb_i == n_hblk - 1)
                        if (dc % 2 == 1 or last_blk) and pending:
                            pending.pop(0)(g, d0)
                    if not (do_tp and do_conv):
                        continue
                    for wh in range(2):
                        tp = tp_pool.tile([128, 3, 1024], F16, tag="tp")
                        w0 = wh * 128
                        for d in range(30):
                            nc.tensor.transpose(
                                tp[:, d // 10,
                                   (d % 10) * 96:(d % 10) * 96 + 96],
                                g[:, d, w0:w0 + 128],
                                idsb[0:96, 0:96])
                        for j in range(2):
                            nc.tensor.transpose(
                                tp[:, j, 960:1008],
                                g[0:48, 30 + j, w0:w0 + 128],
                                idsb[0:48, 0:48])
                        if do_scan:
                            for j in range(2):
                                fresh.extend(
                                    scan_pieces(tp, j, s0 + j * W + w0))
                    pending, fresh = pending + fresh, []
        for p in pending + fresh:
            p()

    nc.finalize()
    return nc


def _host_inputs(x, Wc, b):
    """x: [B, CIN, D, H, W] f32 full input. Returns list of 8 in_maps."""
    bf = np.float16
    # 6 stationaries: idx = t*3+kw, each [128, 96] with cols (j*48+co).
    # x tile: block A (rows 0-47) holds tile-rows (h0, h0+1) at t=0,1;
    # block B (rows 64-111) holds (h0+2, h0+3). Pass t streams A at row
    # h0+t and B at h0+2+t, so taps: A: kh = t - j, B: kh = 2 + t - j.
    wt = np.zeros((NST, 128, 2 * CO), np.float32)
    for t in range(2):
        for kw in range(3):
            idx = t * 3 + kw
            for j in range(2):
                c0 = j * CO
                for blk, khv in ((0, t - j), (64, 2 + t - j)):
                    if khv < 0 or khv > 2:
                        continue
                    for kd in range(3):
                        p0 = blk + kd * 16
                        wt[idx, p0:p0 + 16, c0:c0 + CO] = \
                            Wc[:, :, kd, khv, kw].T
    wt[0, 48, 0:CO] = b
    wt[0, 48, CO:2 * CO] = b
    # 6 j-swap stationaries (for d=30 j=1 gates at rows 0-47): the j=1
    # column block of the main set, as its own M=48 stationary.
    wt2 = np.zeros((NST, 128, CO), np.float32)
    for t in range(2):
        for kw in range(3):
            idx = t * 3 + kw
            for blk, khv in ((0, t - 1), (64, t + 1)):
                if khv < 0 or khv > 2:
                    continue
                for kd in range(3):
                    p0 = blk + kd * 16
                    wt2[idx, p0:p0 + 16, :] = Wc[:, :, kd, khv, kw].T
    wt2[0, 48, :] = b
    wts = np.concatenate(
        [wt.transpose(1, 0, 2).reshape(128, NST * 2 * CO),
         wt2.transpose(1, 0, 2).reshape(128, NST * CO)],
        axis=1).astype(bf)
    assert wts.shape == (128, WCOLS)
    auxa = np.zeros((16, FX), np.float32)
    auxa[0, :] = 1.0
    auxa = auxa.astype(bf)
    idn = np.eye(96, dtype=bf)

    xt = np.ascontiguousarray(x.transpose(1, 2, 0, 3, 4))  # [CIN,D,B,H,W]
    in_maps = []
    for c in range(N_CORES):
        hs, he = c * HSH, (c + 1) * HSH
        xp = np.zeros((CIN, D + 2, B, HSH + 2, WP), np.float32)
        lo = max(hs - 1, 0)
        hi = min(he + 1, H)
        xp[:, 1:D + 1, :, (lo - (hs - 1)):(hi - (hs - 1)), 1:W + 1] = \
            xt[:, :, :, lo:hi, :]
        in_maps.append({"x": xp.astype(bf), "wts": wts, "aux": auxa,
                        "idn": idn})
    return in_maps


_PROGRAM = None


def _get_program():
    global _PROGRAM
    if _PROGRAM is None:
        _PROGRAM = _build_program()
    return _PROGRAM


def run_sharded(in_maps, trace=False, **kw):
    from concourse import bass_utils
    nc = _get_program()
    return bass_utils.run_bass_kernel_spmd(
        nc, in_maps, core_ids=list(range(N_CORES)), trace=trace, **kw)


def _assemble(results):
    outf = np.empty((B, HID, D, H, W), np.float32)
    for c in range(N_CORES):
        raw = np.asarray(results[c]["out"])  # [S, HID, D]
        o = raw.reshape(B, HSH, W, HID, D).transpose(0, 3, 4, 1, 2)
        outf[:, :, :, c * HSH:(c + 1) * HSH, :] = o
    return outf


def kernel(x, W, b):
    x = np.asarray(x, np.float32)
    W = np.asarray(W, np.float32)
    b = np.asarray(b, np.float32)
    in_maps = _host_inputs(x, W, b)
    res = run_sharded(in_maps)
    return _assemble(res.results)


# revision 31
# speedup vs baseline: 1.1188x; 1.1188x over previous
"""Trainium2 Bass kernel for a BiQRNN3D layer.

reference math:
  gates = conv3d(x, W, SAME, 3x3x3) + b          x: [2,16,31,256,256] f32
  Z, F1, F2 = split(gates, 3, channel)           W: [48,16,3,3,3], b: [48]
  Z = tanh(Z); F1 = sigmoid(F1); F2 = sigmoid(F2)
  h_fwd: depth-forward  recurrence h = F1*h + (1-F1)*Z
  h_bwd: depth-backward recurrence h = F2*h + (1-F2)*Z
  out = h_fwd + h_bwd                            [2,16,31,256,256] f32

Distribution: H (=256) is sharded 32 rows per core across 8 NeuronCores
(SPMD, identical program; each core's x shard carries its 1-row conv halo
with global-edge zeros baked in by the host).

Per-core pipeline (v2 - no DRAM gates round-trip):
  * conv as matmul, K = (kd,ci) = 48 contraction rows. The moving x tile
    holds 3 kd-shifted copies in partitions 0-47 (block A) and an
    additional h+1-shifted copy in partitions 64-111 (block B,
    host-prepared). Partition 48 is a ones-row (bias rides as a stationary
    row); partitions 49-63 are zeros.
  * M = 96: stationary columns (j, co) produce BOTH output h rows of an
    h-block at once. Per psum tile [96, 2*256] six K=112 matmuls
    accumulate: passes (p in {0,1}) x (kw in {0,1,2}); pass p streams x
    rows at tile-h 2p, and blocks A/B provide taps kh = 2p-j and 2p+1-j.
  * gates evac psum -> SBUF f16 tile g[96, 32, 256] (d-major). Slot 31
    holds a "j-swap" copy of d=30 for j=1 (6 extra M=48 matmuls with
    j-swapped stationary columns), so both d=30 transposes below read
    from partition base 0.
  * on-chip transpose via PE is_transpose matmuls into PSUM f16 tile
    tp[128, 3 banks, 1024]: per w-half, 30x [96,128] transposes at slot
    (d//10, (d%10)*96) plus 2x [48,128] for d=30 into the 128-f16 bank
    gaps (offset 960). No matmul group crosses a 2KB psum bank.
  * ACT: tanh/sigmoid read strided from psum tp at 128-partition
    utilization; DVE: g = (f-1)*z, tensor_tensor_scan (h = f*h - g) for
    both directions (backward stored d-reversed); f zeroed at d=0 so one
    long scan chains safely across channel runs. out fp32 [S, 16, 31]
    -> host reassembles.
"""

from contextlib import ExitStack

import numpy as np

import concourse.bass as bass
import concourse.tile as tile
from concourse import bacc, mybir

F32 = mybir.dt.float32
F16 = mybir.dt.float16
AF = mybir.ActivationFunctionType
ALU = mybir.AluOpType

N_CORES = 8
B = 2
CIN = 16
HID = 16
CO = 3 * HID            # 48
D = 31
H = 256
W = 256
HSH = H // N_CORES      # 32
HB = 2                  # output h rows per conv tile (= M/CO)
DC = 2                  # d slices per psum tile
WP = W + 2
S = B * HSH * W         # 16384
FX = D * 2 * WP         # x tile free extent per partition
CHUNK = 128
NST = 6                 # stationary matrices (main)
WCOLS = NST * 2 * CO + NST * CO   # 576 + 288 = 864


def _build_program(reps=1, do_conv=True, do_scan=True, do_tp=True):
    nc = bacc.Bacc("TRN2", target_bir_lowering=False, debug=False)

    x_dram = nc.dram_tensor("x", [CIN, D + 2, B, HSH + 2, WP], F16,
                            kind="ExternalInput").ap()
    wts = nc.dram_tensor("wts", [128, WCOLS], F16,
                         kind="ExternalInput").ap()
    aux = nc.dram_tensor("aux", [16, FX], F16, kind="ExternalInput").ap()
    idn = nc.dram_tensor("idn", [96, 96], F16, kind="ExternalInput").ap()
    out = nc.dram_tensor("out", [S, HID, D], F32, kind="ExternalOutput").ap()

    with tile.TileContext(nc) as tc, ExitStack() as ctx:
        wsb = nc.alloc_sbuf_tensor("wsb", [128, WCOLS], F16).ap()
        idsb = nc.alloc_sbuf_tensor("idsb", [96, 96], F16).ap()
        # x tile: A rows hold x at h = h0 + 2t, B rows x at h0 + 1 + 2t
        NXB = 3
        xbufs = [nc.alloc_sbuf_tensor(f"xb{i}", [112, D, 2, WP], F16).ap()
                 for i in range(NXB)]

        nc.sync.dma_start(wsb, wts)
        nc.sync.dma_start(idsb, idn)
        for xb in xbufs:
            nc.sync.dma_start(
                xb[48:64].rearrange("p a b c -> p (a b c)"), aux)

        g_pool = ctx.enter_context(tc.tile_pool(name="gp", bufs=2))
        ps_pool = ctx.enter_context(tc.tile_pool(name="ps", bufs=2,
                                                 space="PSUM"))
        tp_pool = ctx.enter_context(tc.tile_pool(name="tp", bufs=2,
                                                 space="PSUM"))
        sc_pool = ctx.enter_context(tc.tile_pool(name="sc", bufs=10))
        o_pool = ctx.enter_context(tc.tile_pool(name="op", bufs=3))

        n_hblk = HSH // HB
        n_dc = (D + DC - 1) // DC

        def scan_pieces(tp, j, c0):
            """One 128-pixel chunk. Activations are emitted immediately
            (they read the psum tp tile); the post-act SBUF work is
            returned as deferred pieces, drained two h-blocks later so
            its deps are long-satisfied and it fills engine idle time
            without head-of-line blocking the conv-critical evacs.
            f1/f2 live in one ff tile so each sigmoid covers both; f2 is
            stored forward and the backward scan runs on reversed APs."""
            zt = sc_pool.tile([128, HID, D], F16, tag="zt", name="zt")
            ff = sc_pool.tile([128, 2, HID, D], F16, tag="ff", name="ff")
            f1 = ff[:, 0]
            f2 = ff[:, 1]
            # main d 0..29 at psum slot (d//10, (d%10)*96), cols j*48+c
            vm = tp[:, :, 0:960].rearrange("p b (db c) -> p c b db", c=96)
            cb = j * CO
            nc.scalar.activation(
                zt[:, :, 0:30].rearrange("p c (b db) -> p c b db", b=3),
                vm[:, cb:cb + HID], AF.Tanh)
            nc.scalar.activation(
                ff[:, :, :, 0:30].rearrange("p f c (b db) -> p f c b db",
                                            b=3),
                vm[:, cb + HID:cb + 3 * HID].rearrange(
                    "p (f c) b db -> p f c b db", f=2), AF.Sigmoid)
            # d=30 lives in the bank-j gap at offset 960, cols 0-47
            nc.scalar.activation(
                zt[:, :, 30:31],
                tp[:, j:j + 1, 960:960 + HID].rearrange("p a c -> p c a"),
                AF.Tanh)
            nc.scalar.activation(
                ff[:, :, :, 30:31].rearrange("p f c a -> p (f c) a"),
                tp[:, j:j + 1, 960 + HID:960 + 3 * HID]
                .rearrange("p a c -> p c a"), AF.Sigmoid)

            st = {}

            def order_tok(f, dd, gcur, gd0):
                # 1-element self-bypass whose second operand reads the
                # just-evacuated g region: pins this piece AFTER that evac
                # in the scheduler's dependency-topological order.
                if gcur is not None:
                    nc.vector.tensor_tensor(
                        f[0:1, 0:1, dd:dd + 1], f[0:1, 0:1, dd:dd + 1],
                        gcur[0:1, gd0:gd0 + 1, 0:1], ALU.bypass)

            def p1(gcur=None, gd0=0):
                order_tok(f1, 0, gcur, gd0)
                st['g1'] = g1 = sc_pool.tile([128, HID, D], F16, tag="g1",
                                             name="g1")
                nc.vector.scalar_tensor_tensor(
                    g1[:], f1[:], 1.0, zt[:], ALU.subtract, ALU.mult)
                nc.vector.memset(f1[:, :, 0:1], 0.0)
                st['h1'] = h1 = sc_pool.tile([128, HID, D], F16, tag="h1",
                                             name="h1")
                nc.vector.tensor_tensor_scan(
                    h1[:].rearrange("p c d -> p (c d)"),
                    f1[:].rearrange("p c d -> p (c d)"),
                    g1[:].rearrange("p c d -> p (c d)"),
                    0.0, ALU.mult, ALU.subtract)

            def p2(gcur=None, gd0=0):
                order_tok(f2, D - 1, gcur, gd0)
                h1 = st['h1']
                g2 = sc_pool.tile([128, HID, D], F16, tag="g2", name="g2")
                nc.vector.scalar_tensor_tensor(
                    g2[:], f2[:], 1.0, zt[:], ALU.subtract, ALU.mult)
                nc.vector.memset(f2[:, :, D - 1:D], 0.0)
                h2 = sc_pool.tile([128, HID, D], F16, tag="h2", name="h2")
                nc.vector.tensor_tensor_scan(
                    h2[:].rearrange("p c d -> p (c d)")[:, ::-1],
                    f2[:].rearrange("p c d -> p (c d)")[:, ::-1],
                    g2[:].rearrange("p c d -> p (c d)")[:, ::-1],
                    0.0, ALU.mult, ALU.subtract)
                o = o_pool.tile([128, HID, D], F32, tag="o", name="o")
                nc.gpsimd.tensor_add(o[:], h1[:], h2[:])
                nc.gpsimd.dma_start(out[c0:c0 + CHUNK], o[:])

            return [p1, p2]

        tix = 0
        pending = []   # drained two h-blocks later
        fresh = []
        for _rep in range(reps):
            for b_i in range(B):
                for hb_i in range(n_hblk):
                    xb = xbufs[tix % NXB]
                    tix += 1
                    h0 = hb_i * HB
                    for ds0, ds1 in ([(0, 8), (8, 31)] if tix == 1
                                     else [(0, 31)]):
                        for kd in range(3):
                            nc.sync.dma_start(
                                xb[kd * 16:kd * 16 + 16, ds0:ds1].rearrange(
                                    "p d t w -> p d (t w)"),
                                x_dram[:, kd + ds0:kd + ds1, b_i,
                                       h0:h0 + 2, :].rearrange(
                                    "p d t w -> p d (t w)"))
                            nc.sync.dma_start(
                                xb[64 + kd * 16:64 + kd * 16 + 16,
                                   ds0:ds1].rearrange(
                                    "p d t w -> p d (t w)"),
                                x_dram[:, kd + ds0:kd + ds1, b_i,
                                       h0 + 2:h0 + 4, :].rearrange(
                                    "p d t w -> p d (t w)"))
                    s0 = b_i * (HSH * W) + h0 * W
                    g = g_pool.tile([96, 32, W], F16, tag="g")
                    for dc in range(n_dc if do_conv else 0):
                        d0 = dc * DC
                        dn = min(DC, D - d0)
                        ps = ps_pool.tile([96, DC * W], F32, tag="ps")
                        psv = ps[:, 0:dn * W].rearrange(
                            "p (d w) -> p d w", w=W)
                        k = 0
                        for p in range(2):
                            for kw in range(3):
                                nc.tensor.matmul(
                                    psv,
                                    wsb[0:112, k * 96:(k + 1) * 96],
                                    xb[0:112, d0:d0 + dn, p, kw:kw + W],
                                    start=(k == 0), stop=(k == NST - 1))
                                k += 1
                        if dc == n_dc - 1:
                            # j-swap extra: d=30 gates for j=1 at rows 0-47
                            k = 0
                            for p in range(2):
                                for kw in range(3):
                                    nc.tensor.matmul(
                                        ps[0:48, W:2 * W],
                                        wsb[0:112,
                                            576 + k * 48:576 + (k + 1) * 48],
                                        xb[0:112, d0, p, kw:kw + W],
                                        start=(k == 0), stop=(k == NST - 1))
                                    k += 1
                        gv = g[:, d0:d0 + dn, :].rearrange("p d w -> p (d w)")
                        nc.vector.tensor_copy(gv, ps[:, 0:dn * W])
                        if dc == n_dc - 1:
                            nc.vector.tensor_copy(
                                g[0:48, 31, :], ps[0:48, W:2 * W])
                        if dc % 2 == 1 and pending:
                            pending.pop(0)(g, d0)
                    if not (do_tp and do_conv):
                        continue
                    for wh in range(2):
                        tp = tp_pool.tile([128, 3, 1024], F16, tag="tp")
                        w0 = wh * 128
                        for d in range(30):
                            nc.tensor.transpose(
                                tp[:, d // 10,
                                   (d % 10) * 96:(d % 10) * 96 + 96],
                                g[:, d, w0:w0 + 128],
                                idsb[0:96, 0:96])
                        for j in range(2):
                            nc.tensor.transpose(
                                tp[:, j, 960:1008],
                                g[0:48, 30 + j, w0:w0 + 128],
                                idsb[0:48, 0:48])
                        if do_scan:
                            for j in range(2):
                                fresh.extend(
                                    scan_pieces(tp, j, s0 + j * W + w0))
                    pending, fresh = pending + fresh, []
        for p in pending + fresh:
            p()

    nc.finalize()
    return nc


def _host_inputs(x, Wc, b):
    """x: [B, CIN, D, H, W] f32 full input. Returns list of 8 in_maps."""
    bf = np.float16
    # 6 stationaries: idx = t*3+kw, each [128, 96] with cols (j*48+co).
    # x tile: block A (rows 0-47) holds tile-rows (h0, h0+1) at t=0,1;
    # block B (rows 64-111) holds (h0+2, h0+3). Pass t streams A at row
    # h0+t and B at h0+2+t, so taps: A: kh = t - j, B: kh = 2 + t - j.
    wt = np.zeros((NST, 128, 2 * CO), np.float32)
    for t in range(2):
        for kw in range(3):
            idx = t * 3 + kw
            for j in range(2):
                c0 = j * CO
                for blk, khv in ((0, t - j), (64, 2 + t - j)):
                    if khv < 0 or khv > 2:
                        continue
                    for kd in range(3):
                        p0 = blk + kd * 16
                        wt[idx, p0:p0 + 16, c0:c0 + CO] = \
                            Wc[:, :, kd, khv, kw].T
    wt[0, 48, 0:CO] = b
    wt[0, 48, CO:2 * CO] = b
    # 6 j-swap stationaries (for d=30 j=1 gates at rows 0-47): the j=1
    # column block of the main set, as its own M=48 stationary.
    wt2 = np.zeros((NST, 128, CO), np.float32)
    for t in range(2):
        for kw in range(3):
            idx = t * 3 + kw
            for blk, khv in ((0, t - 1), (64, t + 1)):
                if khv < 0 or khv > 2:
                    continue
                for kd in range(3):
                    p0 = blk + kd * 16
                    wt2[idx, p0:p0 + 16, :] = Wc[:, :, kd, khv, kw].T
    wt2[0, 48, :] = b
    wts = np.concatenate(
        [wt.transpose(1, 0, 2).reshape(128, NST * 2 * CO),
         wt2.transpose(1, 0, 2).reshape(128, NST * CO)],
        axis=1).astype(bf)
    assert wts.shape == (128, WCOLS)
    auxa = np.zeros((16, FX), np.float32)
    auxa[0, :] = 1.0
    auxa = auxa.astype(bf)
    idn = np.eye(96, dtype=bf)

    xt = np.ascontiguousarray(x.transpose(1, 2, 0, 3, 4))  # [CIN,D,B,H,W]
    in_maps = []
    for c in range(N_CORES):
        hs, he = c * HSH, (c + 1) * HSH
        xp = np.zeros((CIN, D + 2, B, HSH + 2, WP), np.float32)
        lo = max(hs - 1, 0)
        hi = min(he + 1, H)
        xp[:, 1:D + 1, :, (lo - (hs - 1)):(hi - (hs - 1)), 1:W + 1] = \
            xt[:, :, :, lo:hi, :]
        in_maps.append({"x": xp.astype(bf), "wts": wts, "aux": auxa,
                        "idn": idn})
    return in_maps


_PROGRAM = None


def _get_program():
    global _PROGRAM
    if _PROGRAM is None:
        _PROGRAM = _build_program()
    return _PROGRAM


def run_sharded(in_maps, trace=False, **kw):
    from concourse import bass_utils
    nc = _get_program()
    return bass_utils.run_bass_kernel_spmd(
        nc, in_maps, core_ids=list(range(N_CORES)), trace=trace, **kw)


def _assemble(results):
    outf = np.empty((B, HID, D, H, W), np.float32)
    for c in range(N_CORES):
        raw = np.asarray(results[c]["out"])  # [S, HID, D]
        o = raw.reshape(B, HSH, W, HID, D).transpose(0, 3, 4, 1, 2)
        outf[:, :, :, c * HSH:(c + 1) * HSH, :] = o
    return outf


def kernel(x, W, b):
    x = np.asarray(x, np.float32)
    W = np.asarray(W, np.float32)
    b = np.asarray(b, np.float32)
    in_maps = _host_inputs(x, W, b)
    res = run_sharded(in_maps)
    return _assemble(res.results)


# revision 32
# speedup vs baseline: 1.1672x; 1.0433x over previous
"""Trainium2 Bass kernel for a BiQRNN3D layer.

reference math:
  gates = conv3d(x, W, SAME, 3x3x3) + b          x: [2,16,31,256,256] f32
  Z, F1, F2 = split(gates, 3, channel)           W: [48,16,3,3,3], b: [48]
  Z = tanh(Z); F1 = sigmoid(F1); F2 = sigmoid(F2)
  h_fwd: depth-forward  recurrence h = F1*h + (1-F1)*Z
  h_bwd: depth-backward recurrence h = F2*h + (1-F2)*Z
  out = h_fwd + h_bwd                            [2,16,31,256,256] f32

Distribution: H (=256) is sharded 32 rows per core across 8 NeuronCores
(SPMD, identical program; each core's x shard carries its 1-row conv halo
with global-edge zeros baked in by the host).

Per-core pipeline (v2 - no DRAM gates round-trip):
  * conv as matmul, K = (kd,ci) = 48 contraction rows. The moving x tile
    holds 3 kd-shifted copies in partitions 0-47 (block A) and an
    additional h+1-shifted copy in partitions 64-111 (block B,
    host-prepared). Partition 48 is a ones-row (bias rides as a stationary
    row); partitions 49-63 are zeros.
  * M = 96: stationary columns (j, co) produce BOTH output h rows of an
    h-block at once. Per psum tile [96, 2*256] six K=112 matmuls
    accumulate: passes (p in {0,1}) x (kw in {0,1,2}); pass p streams x
    rows at tile-h 2p, and blocks A/B provide taps kh = 2p-j and 2p+1-j.
  * gates evac psum -> SBUF f16 tile g[96, 32, 256] (d-major). Slot 31
    holds a "j-swap" copy of d=30 for j=1 (6 extra M=48 matmuls with
    j-swapped stationary columns), so both d=30 transposes below read
    from partition base 0.
  * on-chip transpose via PE is_transpose matmuls into PSUM f16 tile
    tp[128, 3 banks, 1024]: per w-half, 30x [96,128] transposes at slot
    (d//10, (d%10)*96) plus 2x [48,128] for d=30 into the 128-f16 bank
    gaps (offset 960). No matmul group crosses a 2KB psum bank.
  * ACT: tanh/sigmoid read strided from psum tp at 128-partition
    utilization; DVE: g = (f-1)*z, tensor_tensor_scan (h = f*h - g) for
    both directions (backward stored d-reversed); f zeroed at d=0 so one
    long scan chains safely across channel runs. out fp32 [S, 16, 31]
    -> host reassembles.
"""

from contextlib import ExitStack

import numpy as np

import concourse.bass as bass
import concourse.tile as tile
from concourse import bacc, mybir

F32 = mybir.dt.float32
F16 = mybir.dt.float16
AF = mybir.ActivationFunctionType
ALU = mybir.AluOpType

N_CORES = 8
B = 2
CIN = 16
HID = 16
CO = 3 * HID            # 48
D = 31
H = 256
W = 256
HSH = H // N_CORES      # 32
HB = 2                  # output h rows per conv tile (= M/CO)
DC = 2                  # d slices per psum tile
WP = W + 2
S = B * HSH * W         # 16384
FX = D * 2 * WP         # x tile free extent per partition
CHUNK = 128
NST = 6                 # stationary matrices (main)
WCOLS = NST * 2 * CO + NST * CO   # 576 + 288 = 864


def _build_program(reps=1, do_conv=True, do_scan=True, do_tp=True):
    nc = bacc.Bacc("TRN2", target_bir_lowering=False, debug=False)

    x_dram = nc.dram_tensor("x", [CIN, D + 2, B, HSH + 2, WP], F16,
                            kind="ExternalInput").ap()
    wts = nc.dram_tensor("wts", [128, WCOLS], F16,
                         kind="ExternalInput").ap()
    aux = nc.dram_tensor("aux", [16, FX], F16, kind="ExternalInput").ap()
    idn = nc.dram_tensor("idn", [96, 96], F16, kind="ExternalInput").ap()
    out = nc.dram_tensor("out", [S, HID, D], F32, kind="ExternalOutput").ap()

    with tile.TileContext(nc) as tc, ExitStack() as ctx:
        wsb = nc.alloc_sbuf_tensor("wsb", [128, WCOLS], F16).ap()
        idsb = nc.alloc_sbuf_tensor("idsb", [96, 96], F16).ap()
        # x tile: A rows hold x at h = h0 + 2t, B rows x at h0 + 1 + 2t
        NXB = 2
        xbufs = [nc.alloc_sbuf_tensor(f"xb{i}", [112, D, 2, WP], F16).ap()
                 for i in range(NXB)]

        nc.sync.dma_start(wsb, wts)
        nc.sync.dma_start(idsb, idn)
        for xb in xbufs:
            nc.sync.dma_start(
                xb[48:64].rearrange("p a b c -> p (a b c)"), aux)

        g_pool = ctx.enter_context(tc.tile_pool(name="gp", bufs=2))
        ps_pool = ctx.enter_context(tc.tile_pool(name="ps", bufs=2,
                                                 space="PSUM"))
        tp_pool = ctx.enter_context(tc.tile_pool(name="tp", bufs=2,
                                                 space="PSUM"))
        sc_pool = ctx.enter_context(tc.tile_pool(name="sc", bufs=12))
        o_pool = ctx.enter_context(tc.tile_pool(name="op", bufs=3))

        n_hblk = HSH // HB
        n_dc = (D + DC - 1) // DC

        def scan_pieces(tp, j, c0):
            """One 128-pixel chunk. Activations are emitted immediately
            (they read the psum tp tile); the post-act SBUF work is
            returned as deferred pieces, drained two h-blocks later so
            its deps are long-satisfied and it fills engine idle time
            without head-of-line blocking the conv-critical evacs.
            f1/f2 live in one ff tile so each sigmoid covers both; f2 is
            stored forward and the backward scan runs on reversed APs."""
            zt = sc_pool.tile([128, HID, D], F16, tag="zt", name="zt")
            ff = sc_pool.tile([128, 2, HID, D], F16, tag="ff", name="ff")
            f1 = ff[:, 0]
            f2 = ff[:, 1]
            # main d 0..29 at psum slot (d//10, (d%10)*96), cols j*48+c
            vm = tp[:, :, 0:960].rearrange("p b (db c) -> p c b db", c=96)
            cb = j * CO
            nc.scalar.activation(
                zt[:, :, 0:30].rearrange("p c (b db) -> p c b db", b=3),
                vm[:, cb:cb + HID], AF.Tanh)
            nc.scalar.activation(
                ff[:, :, :, 0:30].rearrange("p f c (b db) -> p f c b db",
                                            b=3),
                vm[:, cb + HID:cb + 3 * HID].rearrange(
                    "p (f c) b db -> p f c b db", f=2), AF.Sigmoid)
            # d=30 lives in the bank-j gap at offset 960, cols 0-47
            nc.scalar.activation(
                zt[:, :, 30:31],
                tp[:, j:j + 1, 960:960 + HID].rearrange("p a c -> p c a"),
                AF.Tanh)
            nc.scalar.activation(
                ff[:, :, :, 30:31].rearrange("p f c a -> p (f c) a"),
                tp[:, j:j + 1, 960 + HID:960 + 3 * HID]
                .rearrange("p a c -> p c a"), AF.Sigmoid)

            st = {}

            def order_tok(f, dd, gcur, gd0):
                # 1-element self-bypass whose second operand reads the
                # just-evacuated g region: pins this piece AFTER that evac
                # in the scheduler's dependency-topological order.
                if gcur is not None:
                    nc.vector.tensor_tensor(
                        f[0:1, 0:1, dd:dd + 1], f[0:1, 0:1, dd:dd + 1],
                        gcur[0:1, gd0:gd0 + 1, 0:1], ALU.bypass)

            def p1(gcur=None, gd0=0):
                order_tok(f1, 0, gcur, gd0)
                st['g1'] = g1 = sc_pool.tile([128, HID, D], F16, tag="g1",
                                             name="g1")
                nc.vector.scalar_tensor_tensor(
                    g1[:], f1[:], 1.0, zt[:], ALU.subtract, ALU.mult)
                nc.vector.memset(f1[:, :, 0:1], 0.0)
                st['h1'] = h1 = sc_pool.tile([128, HID, D], F16, tag="h1",
                                             name="h1")
                nc.vector.tensor_tensor_scan(
                    h1[:].rearrange("p c d -> p (c d)"),
                    f1[:].rearrange("p c d -> p (c d)"),
                    g1[:].rearrange("p c d -> p (c d)"),
                    0.0, ALU.mult, ALU.subtract)

            def p2(gcur=None, gd0=0):
                order_tok(f2, D - 1, gcur, gd0)
                h1 = st['h1']
                g2 = sc_pool.tile([128, HID, D], F16, tag="g2", name="g2")
                nc.vector.scalar_tensor_tensor(
                    g2[:], f2[:], 1.0, zt[:], ALU.subtract, ALU.mult)
                nc.vector.memset(f2[:, :, D - 1:D], 0.0)
                h2 = sc_pool.tile([128, HID, D], F16, tag="h2", name="h2")
                nc.vector.tensor_tensor_scan(
                    h2[:].rearrange("p c d -> p (c d)")[:, ::-1],
                    f2[:].rearrange("p c d -> p (c d)")[:, ::-1],
                    g2[:].rearrange("p c d -> p (c d)")[:, ::-1],
                    0.0, ALU.mult, ALU.subtract)
                o = o_pool.tile([128, HID, D], F32, tag="o", name="o")
                nc.gpsimd.tensor_add(o[:], h1[:], h2[:])
                nc.gpsimd.dma_start(out[c0:c0 + CHUNK], o[:])

            return [p1, p2]

        tix = 0
        pending = []   # drained two h-blocks later
        fresh = []
        for _rep in range(reps):
            for b_i in range(B):
                for hb_i in range(n_hblk):
                    xb = xbufs[tix % NXB]
                    tix += 1
                    h0 = hb_i * HB
                    for ds0, ds1 in ([(0, 8), (8, 31)] if tix == 1
                                     else [(0, 31)]):
                        for kd in range(3):
                            nc.sync.dma_start(
                                xb[kd * 16:kd * 16 + 16, ds0:ds1].rearrange(
                                    "p d t w -> p d (t w)"),
                                x_dram[:, kd + ds0:kd + ds1, b_i,
                                       h0:h0 + 2, :].rearrange(
                                    "p d t w -> p d (t w)"))
                            nc.sync.dma_start(
                                xb[64 + kd * 16:64 + kd * 16 + 16,
                                   ds0:ds1].rearrange(
                                    "p d t w -> p d (t w)"),
                                x_dram[:, kd + ds0:kd + ds1, b_i,
                                       h0 + 2:h0 + 4, :].rearrange(
                                    "p d t w -> p d (t w)"))
                    s0 = b_i * (HSH * W) + h0 * W
                    g = g_pool.tile([96, 32, W], F16, tag="g")
                    for dc in range(n_dc if do_conv else 0):
                        d0 = dc * DC
                        dn = min(DC, D - d0)
                        ps = ps_pool.tile([96, DC * W], F32, tag="ps")
                        psv = ps[:, 0:dn * W].rearrange(
                            "p (d w) -> p d w", w=W)
                        k = 0
                        for p in range(2):
                            for kw in range(3):
                                nc.tensor.matmul(
                                    psv,
                                    wsb[0:112, k * 96:(k + 1) * 96],
                                    xb[0:112, d0:d0 + dn, p, kw:kw + W],
                                    start=(k == 0), stop=(k == NST - 1))
                                k += 1
                        if dc == n_dc - 1:
                            # j-swap extra: d=30 gates for j=1 at rows 0-47
                            k = 0
                            for p in range(2):
                                for kw in range(3):
                                    nc.tensor.matmul(
                                        ps[0:48, W:2 * W],
                                        wsb[0:112,
                                            576 + k * 48:576 + (k + 1) * 48],
                                        xb[0:112, d0, p, kw:kw + W],
                                        start=(k == 0), stop=(k == NST - 1))
                                    k += 1
                        gv = g[:, d0:d0 + dn, :].rearrange("p d w -> p (d w)")
                        nc.vector.tensor_copy(gv, ps[:, 0:dn * W])
                        if dc == n_dc - 1:
                            nc.vector.tensor_copy(
                                g[0:48, 31, :], ps[0:48, W:2 * W])
                        if dc % 2 == 1 and pending:
                            pending.pop(0)(g, d0)
                    if not (do_tp and do_conv):
                        continue
                    for wh in range(2):
                        tp = tp_pool.tile([128, 3, 1024], F16, tag="tp")
                        w0 = wh * 128
                        for d in range(30):
                            nc.tensor.transpose(
                                tp[:, d // 10,
                                   (d % 10) * 96:(d % 10) * 96 + 96],
                                g[:, d, w0:w0 + 128],
                                idsb[0:96, 0:96])
                        for j in range(2):
                            nc.tensor.transpose(
                                tp[:, j, 960:1008],
                                g[0:48, 30 + j, w0:w0 + 128],
                                idsb[0:48, 0:48])
                        if do_scan:
                            for j in range(2):
                                fresh.extend(
                                    scan_pieces(tp, j, s0 + j * W + w0))
                    pending, fresh = pending + fresh, []
        for p in pending + fresh:
            p()

    nc.finalize()
    return nc


def _host_inputs(x, Wc, b):
    """x: [B, CIN, D, H, W] f32 full input. Returns list of 8 in_maps."""
    bf = np.float16
    # 6 stationaries: idx = t*3+kw, each [128, 96] with cols (j*48+co).
    # x tile: block A (rows 0-47) holds tile-rows (h0, h0+1) at t=0,1;
    # block B (rows 64-111) holds (h0+2, h0+3). Pass t streams A at row
    # h0+t and B at h0+2+t, so taps: A: kh = t - j, B: kh = 2 + t - j.
    wt = np.zeros((NST, 128, 2 * CO), np.float32)
    for t in range(2):
        for kw in range(3):
            idx = t * 3 + kw
            for j in range(2):
                c0 = j * CO
                for blk, khv in ((0, t - j), (64, 2 + t - j)):
                    if khv < 0 or khv > 2:
                        continue
                    for kd in range(3):
                        p0 = blk + kd * 16
                        wt[idx, p0:p0 + 16, c0:c0 + CO] = \
                            Wc[:, :, kd, khv, kw].T
    wt[0, 48, 0:CO] = b
    wt[0, 48, CO:2 * CO] = b
    # 6 j-swap stationaries (for d=30 j=1 gates at rows 0-47): the j=1
    # column block of the main set, as its own M=48 stationary.
    wt2 = np.zeros((NST, 128, CO), np.float32)
    for t in range(2):
        for kw in range(3):
            idx = t * 3 + kw
            for blk, khv in ((0, t - 1), (64, t + 1)):
                if khv < 0 or khv > 2:
                    continue
                for kd in range(3):
                    p0 = blk + kd * 16
                    wt2[idx, p0:p0 + 16, :] = Wc[:, :, kd, khv, kw].T
    wt2[0, 48, :] = b
    wts = np.concatenate(
        [wt.transpose(1, 0, 2).reshape(128, NST * 2 * CO),
         wt2.transpose(1, 0, 2).reshape(128, NST * CO)],
        axis=1).astype(bf)
    assert wts.shape == (128, WCOLS)
    auxa = np.zeros((16, FX), np.float32)
    auxa[0, :] = 1.0
    auxa = auxa.astype(bf)
    idn = np.eye(96, dtype=bf)

    xt = np.ascontiguousarray(x.transpose(1, 2, 0, 3, 4))  # [CIN,D,B,H,W]
    in_maps = []
    for c in range(N_CORES):
        hs, he = c * HSH, (c + 1) * HSH
        xp = np.zeros((CIN, D + 2, B, HSH + 2, WP), np.float32)
        lo = max(hs - 1, 0)
        hi = min(he + 1, H)
        xp[:, 1:D + 1, :, (lo - (hs - 1)):(hi - (hs - 1)), 1:W + 1] = \
            xt[:, :, :, lo:hi, :]
        in_maps.append({"x": xp.astype(bf), "wts": wts, "aux": auxa,
                        "idn": idn})
    return in_maps


_PROGRAM = None


def _get_program():
    global _PROGRAM
    if _PROGRAM is None:
        _PROGRAM = _build_program()
    return _PROGRAM


def run_sharded(in_maps, trace=False, **kw):
    from concourse import bass_utils
    nc = _get_program()
    return bass_utils.run_bass_kernel_spmd(
        nc, in_maps, core_ids=list(range(N_CORES)), trace=trace, **kw)


def _assemble(results):
    outf = np.empty((B, HID, D, H, W), np.float32)
    for c in range(N_CORES):
        raw = np.asarray(results[c]["out"])  # [S, HID, D]
        o = raw.reshape(B, HSH, W, HID, D).transpose(0, 3, 4, 1, 2)
        outf[:, :, :, c * HSH:(c + 1) * HSH, :] = o
    return outf


def kernel(x, W, b):
    x = np.asarray(x, np.float32)
    W = np.asarray(W, np.float32)
    b = np.asarray(b, np.float32)
    in_maps = _host_inputs(x, W, b)
    res = run_sharded(in_maps)
    return _assemble(res.results)


# revision 38
# speedup vs baseline: 1.2094x; 1.0362x over previous
"""Trainium2 Bass kernel for a BiQRNN3D layer.

reference math:
  gates = conv3d(x, W, SAME, 3x3x3) + b          x: [2,16,31,256,256] f32
  Z, F1, F2 = split(gates, 3, channel)           W: [48,16,3,3,3], b: [48]
  Z = tanh(Z); F1 = sigmoid(F1); F2 = sigmoid(F2)
  h_fwd: depth-forward  recurrence h = F1*h + (1-F1)*Z
  h_bwd: depth-backward recurrence h = F2*h + (1-F2)*Z
  out = h_fwd + h_bwd                            [2,16,31,256,256] f32

Distribution: H (=256) is sharded 32 rows per core across 8 NeuronCores
(SPMD, identical program; each core's x shard carries its 1-row conv halo
with global-edge zeros baked in by the host).

Per-core pipeline (fully on-chip, no DRAM round-trip for gates):
  * conv as matmul, K = (kd,ci) = 48 contraction rows. The moving x tile
    holds 3 kd-shifted copies: block A (partitions 0-47) = tile rows
    (h0, h0+1) at t=0,1; block B (64-111) = rows (h0+2, h0+3); so pass t
    taps are kh = t-j (A) and kh = 2+t-j (B). Partition 48 is a ones-row
    (bias rides as a stationary row).
  * M = 96: stationary columns (j, co) produce BOTH output h rows of an
    h-block at once. Per psum tile [96, 2*256] six K=112 matmuls
    accumulate over passes (t in {0,1}) x (kw in {0,1,2}).
  * gates evac psum -> SBUF f16 tile g[112, 31, 256] (d-major) on DVE.
    The last d slice uses a dedicated 112-wide stationary set (j1 at
    cols 64-111) so the d=30 j1 transpose reads partition base 64 -- a
    legal PE tile position -- with no extra matmuls.
  * on-chip transpose via PE is_transpose matmuls into PSUM f16 tile
    tp[128, 3 banks, 1024]: per w-half, 30x [96,128] transposes at slot
    (d//10, (d%10)*96) plus 2x [48,128] for d=30 into the 128-f16 bank
    gaps (offset 960). No matmul group crosses a 2KB psum bank.
  * ACT reads strided from psum tp at 128-partition utilization: one
    tanh + one double-sigmoid (f1+f2 share one ff tile) for d 0..29,
    plus two small ops for d=30. f2 is stored forward; the backward
    scan runs on fully-reversed APs.
  * DVE: evac copies + g = (f-1)*z + f zeroed at the chain starts +
    tensor_tensor_scan (h = f*h - g) both directions; Pool: h1+h2 and
    the output DMA. out fp32 [S, 16, 31] -> host reassembles.

Scheduling: the Tile scheduler orders instructions topologically, so the
scan-stage work is split into deferred "pieces" drained two h-blocks
late, each pinned behind the current block's evac stream by a 1-element
bypass op reading the just-evacuated g region. This keeps the
conv-critical psum evacs at the head of the DVE queue and makes the
scan work pure idle-time filler; the kernel is then PE-bound
(sim: conv 655us + transposes 82us of 763us total per core).
"""

from contextlib import ExitStack

import numpy as np

import concourse.bass as bass
import concourse.tile as tile
from concourse import bacc, mybir

F32 = mybir.dt.float32
F16 = mybir.dt.float16
AF = mybir.ActivationFunctionType
ALU = mybir.AluOpType

N_CORES = 8
B = 2
CIN = 16
HID = 16
CO = 3 * HID            # 48
D = 31
H = 256
W = 256
HSH = H // N_CORES      # 32
HB = 2                  # output h rows per conv tile (= M/CO)
DC = 2                  # d slices per psum tile
WP = W + 2
S = B * HSH * W         # 16384
FX = D * 2 * WP         # x tile free extent per partition
CHUNK = 128
NST = 6                 # stationary matrices (main)
WCOLS = NST * 2 * CO + NST * 112  # 576 + 672 = 1248


def _build_program(reps=1, do_conv=True, do_scan=True, do_tp=True):
    nc = bacc.Bacc("TRN2", target_bir_lowering=False, debug=False)

    x_dram = nc.dram_tensor("x", [CIN, D + 2, B, HSH + 2, WP], F16,
                            kind="ExternalInput").ap()
    wts = nc.dram_tensor("wts", [128, WCOLS], F16,
                         kind="ExternalInput").ap()
    aux = nc.dram_tensor("aux", [16, FX], F16, kind="ExternalInput").ap()
    idn = nc.dram_tensor("idn", [128, 144], F16,
                     kind="ExternalInput").ap()
    out = nc.dram_tensor("out", [S, HID, D], F32, kind="ExternalOutput").ap()

    with tile.TileContext(nc) as tc, ExitStack() as ctx:
        wsb = nc.alloc_sbuf_tensor("wsb", [128, WCOLS], F16).ap()
        idsb = nc.alloc_sbuf_tensor("idsb", [128, 144], F16).ap()
        # x tile: block A rows hold tile-rows (h0, h0+1) at t=0,1;
        # block B rows hold (h0+2, h0+3).
        NXB = 2
        xbufs = [nc.alloc_sbuf_tensor(f"xb{i}", [112, D, 2, WP], F16).ap()
                 for i in range(NXB)]

        nc.sync.dma_start(wsb, wts)
        nc.sync.dma_start(idsb, idn)
        for xb in xbufs:
            nc.sync.dma_start(
                xb[48:64].rearrange("p a b c -> p (a b c)"), aux)

        g_pool = ctx.enter_context(tc.tile_pool(name="gp", bufs=2))
        ps_pool = ctx.enter_context(tc.tile_pool(name="ps", bufs=2,
                                                 space="PSUM"))
        tp_pool = ctx.enter_context(tc.tile_pool(name="tp", bufs=2,
                                                 space="PSUM"))
        sc_pool = ctx.enter_context(tc.tile_pool(name="sc", bufs=12))
        o_pool = ctx.enter_context(tc.tile_pool(name="op", bufs=3))

        n_hblk = HSH // HB
        n_dc = (D + DC - 1) // DC

        def scan_pieces(tp, j, c0):
            """One 128-pixel chunk. Activations are emitted immediately
            (they read the psum tp tile); the post-act SBUF work is
            returned as deferred pieces, drained two h-blocks later so
            its deps are long-satisfied and it fills engine idle time
            without head-of-line blocking the conv-critical evacs.
            f1/f2 live in one ff tile so each sigmoid covers both; f2 is
            stored forward and the backward scan runs on reversed APs."""
            zt = sc_pool.tile([128, HID, D], F16, tag="zt", name="zt")
            ff = sc_pool.tile([128, 2, HID, D], F16, tag="ff", name="ff")
            f1 = ff[:, 0]
            f2 = ff[:, 1]
            # main d 0..29 at psum slot (d//10, (d%10)*96), cols j*48+c
            vm = tp[:, :, 0:960].rearrange("p b (db c) -> p c b db", c=96)
            cb = j * CO
            nc.scalar.activation(
                zt[:, :, 0:30].rearrange("p c (b db) -> p c b db", b=3),
                vm[:, cb:cb + HID], AF.Tanh)
            nc.scalar.activation(
                ff[:, :, :, 0:30].rearrange("p f c (b db) -> p f c b db",
                                            b=3),
                vm[:, cb + HID:cb + 3 * HID].rearrange(
                    "p (f c) b db -> p f c b db", f=2), AF.Sigmoid)
            # d=30 lives in the bank-j gap at offset 960, cols 0-47
            nc.scalar.activation(
                zt[:, :, 30:31],
                tp[:, j:j + 1, 960:960 + HID].rearrange("p a c -> p c a"),
                AF.Tanh)
            nc.scalar.activation(
                ff[:, :, :, 30:31].rearrange("p f c a -> p (f c) a"),
                tp[:, j:j + 1, 960 + HID:960 + 3 * HID]
                .rearrange("p a c -> p c a"), AF.Sigmoid)

            st = {}

            def order_tok(f, dd, gcur, gd0):
                # 1-element self-bypass whose second operand reads the
                # just-evacuated g region: pins this piece AFTER that evac
                # in the scheduler's dependency-topological order.
                if gcur is not None:
                    nc.vector.tensor_tensor(
                        f[0:1, 0:1, dd:dd + 1], f[0:1, 0:1, dd:dd + 1],
                        gcur[0:1, gd0:gd0 + 1, 0:1], ALU.bypass)

            def p1(gcur=None, gd0=0):
                order_tok(f1, 0, gcur, gd0)
                st['g1'] = g1 = sc_pool.tile([128, HID, D], F16, tag="g1",
                                             name="g1")
                nc.vector.scalar_tensor_tensor(
                    g1[:], f1[:], 1.0, zt[:], ALU.subtract, ALU.mult)
                nc.vector.memset(f1[:, :, 0:1], 0.0)
                st['h1'] = h1 = sc_pool.tile([128, HID, D], F16, tag="h1",
                                             name="h1")
                nc.vector.tensor_tensor_scan(
                    h1[:].rearrange("p c d -> p (c d)"),
                    f1[:].rearrange("p c d -> p (c d)"),
                    g1[:].rearrange("p c d -> p (c d)"),
                    0.0, ALU.mult, ALU.subtract)

            def p2(gcur=None, gd0=0):
                order_tok(f2, D - 1, gcur, gd0)
                h1 = st['h1']
                g2 = sc_pool.tile([128, HID, D], F16, tag="g2", name="g2")
                nc.vector.scalar_tensor_tensor(
                    g2[:], f2[:], 1.0, zt[:], ALU.subtract, ALU.mult)
                nc.vector.memset(f2[:, :, D - 1:D], 0.0)
                h2 = sc_pool.tile([128, HID, D], F16, tag="h2", name="h2")
                nc.vector.tensor_tensor_scan(
                    h2[:].rearrange("p c d -> p (c d)")[:, ::-1],
                    f2[:].rearrange("p c d -> p (c d)")[:, ::-1],
                    g2[:].rearrange("p c d -> p (c d)")[:, ::-1],
                    0.0, ALU.mult, ALU.subtract)
                o = o_pool.tile([128, HID, D], F32, tag="o", name="o")
                nc.gpsimd.tensor_add(o[:], h1[:], h2[:])
                nc.gpsimd.dma_start(out[c0:c0 + CHUNK], o[:])

            return [p1, p2]

        tix = 0
        pending = []   # drained two h-blocks later
        fresh = []
        for _rep in range(reps):
            for b_i in range(B):
                for hb_i in range(n_hblk):
                    xb = xbufs[tix % NXB]
                    tix += 1
                    h0 = hb_i * HB
                    for ds0, ds1 in ([(0, 8), (8, 31)] if tix == 1
                                     else [(0, 31)]):
                        for kd in range(3):
                            nc.sync.dma_start(
                                xb[kd * 16:kd * 16 + 16, ds0:ds1].rearrange(
                                    "p d t w -> p d (t w)"),
                                x_dram[:, kd + ds0:kd + ds1, b_i,
                                       h0:h0 + 2, :].rearrange(
                                    "p d t w -> p d (t w)"))
                            nc.sync.dma_start(
                                xb[64 + kd * 16:64 + kd * 16 + 16,
                                   ds0:ds1].rearrange(
                                    "p d t w -> p d (t w)"),
                                x_dram[:, kd + ds0:kd + ds1, b_i,
                                       h0 + 2:h0 + 4, :].rearrange(
                                    "p d t w -> p d (t w)"))
                    s0 = b_i * (HSH * W) + h0 * W
                    g = g_pool.tile([112, D, W], F16, tag="g")
                    for dc in range(n_dc if do_conv else 0):
                        d0 = dc * DC
                        dn = min(DC, D - d0)
                        ps = ps_pool.tile([112, DC * W], F32, tag="ps")
                        last = dc == n_dc - 1
                        # the last d slice uses a 112-wide stationary set
                        # (j1 at cols 64-111) so the d=30 j1 transpose can
                        # read partition base 64 directly.
                        mw = 112 if last else 96
                        psv = ps[0:mw, 0:dn * W].rearrange(
                            "p (d w) -> p d w", w=W)
                        k = 0
                        for p in range(2):
                            for kw in range(3):
                                if last:
                                    st_ap = wsb[0:112,
                                                576 + k * 112:
                                                576 + (k + 1) * 112]
                                else:
                                    st_ap = wsb[0:112, k * 96:(k + 1) * 96]
                                nc.tensor.matmul(
                                    psv, st_ap,
                                    xb[0:112, d0:d0 + dn, p, kw:kw + W],
                                    start=(k == 0), stop=(k == NST - 1))
                                k += 1
                        gv = g[0:mw, d0:d0 + dn, :].rearrange(
                            "p d w -> p (d w)")
                        nc.vector.tensor_copy(gv, ps[0:mw, 0:dn * W])
                        if dc % 2 == 1 and pending:
                            pending.pop(0)(g, d0)
                    if not (do_tp and do_conv):
                        continue
                    for wh in range(2):
                        tp = tp_pool.tile([128, 3, 1024], F16, tag="tp")
                        w0 = wh * 128
                        for d in range(30):
                            nc.tensor.transpose(
                                tp[:, d // 10,
                                   (d % 10) * 96:(d % 10) * 96 + 96],
                                g[0:96, d, w0:w0 + 128],
                                idsb[0:96, 0:96])
                        nc.tensor.transpose(
                            tp[:, 0, 960:1008],
                            g[0:48, 30, w0:w0 + 128],
                            idsb[0:48, 0:48])
                        nc.tensor.transpose(
                            tp[:, 1, 960:1008],
                            g[64:112, 30, w0:w0 + 128],
                            idsb[64:112, 96:144])
                        if do_scan:
                            for j in range(2):
                                fresh.extend(
                                    scan_pieces(tp, j, s0 + j * W + w0))
                    pending, fresh = pending + fresh, []
        for p in pending + fresh:
            p()

    nc.finalize()
    return nc


def _host_inputs(x, Wc, b):
    """x: [B, CIN, D, H, W] f32 full input. Returns list of 8 in_maps."""
    bf = np.float16
    # 6 stationaries: idx = t*3+kw, each [128, 96] with cols (j*48+co).
    # x tile: block A (rows 0-47) holds tile-rows (h0, h0+1) at t=0,1;
    # block B (rows 64-111) holds (h0+2, h0+3). Pass t streams A at row
    # h0+t and B at h0+2+t, so taps: A: kh = t - j, B: kh = 2 + t - j.
    wt = np.zeros((NST, 128, 2 * CO), np.float32)
    for t in range(2):
        for kw in range(3):
            idx = t * 3 + kw
            for j in range(2):
                c0 = j * CO
                for blk, khv in ((0, t - j), (64, 2 + t - j)):
                    if khv < 0 or khv > 2:
                        continue
                    for kd in range(3):
                        p0 = blk + kd * 16
                        wt[idx, p0:p0 + 16, c0:c0 + CO] = \
                            Wc[:, :, kd, khv, kw].T
    wt[0, 48, 0:CO] = b
    wt[0, 48, CO:2 * CO] = b
    # 6 stationaries for the last d slice, 112 wide: j0 cols 0-47,
    # j1 cols 64-111 (so the d=30 j1 transpose reads base partition 64).
    wt3 = np.zeros((NST, 128, 112), np.float32)
    for t in range(2):
        for kw in range(3):
            idx = t * 3 + kw
            for j in range(2):
                c0 = j * 64
                for blk, khv in ((0, t - j), (64, 2 + t - j)):
                    if khv < 0 or khv > 2:
                        continue
                    for kd in range(3):
                        p0 = blk + kd * 16
                        wt3[idx, p0:p0 + 16, c0:c0 + CO] = \
                            Wc[:, :, kd, khv, kw].T
    wt3[0, 48, 0:CO] = b
    wt3[0, 48, 64:64 + CO] = b
    wts = np.concatenate(
        [wt.transpose(1, 0, 2).reshape(128, NST * 2 * CO),
         wt3.transpose(1, 0, 2).reshape(128, NST * 112)],
        axis=1).astype(bf)
    assert wts.shape == (128, WCOLS)
    auxa = np.zeros((16, FX), np.float32)
    auxa[0, :] = 1.0
    auxa = auxa.astype(bf)
    idn = np.zeros((128, 144), np.float32)
    idn[0:96, 0:96] = np.eye(96)
    idn[64:112, 96:144] = np.eye(48)
    idn = idn.astype(bf)

    xt = np.ascontiguousarray(x.transpose(1, 2, 0, 3, 4))  # [CIN,D,B,H,W]
    in_maps = []
    for c in range(N_CORES):
        hs, he = c * HSH, (c + 1) * HSH
        xp = np.zeros((CIN, D + 2, B, HSH + 2, WP), np.float32)
        lo = max(hs - 1, 0)
        hi = min(he + 1, H)
        xp[:, 1:D + 1, :, (lo - (hs - 1)):(hi - (hs - 1)), 1:W + 1] = \
            xt[:, :, :, lo:hi, :]
        in_maps.append({"x": xp.astype(bf), "wts": wts, "aux": auxa,
                        "idn": idn})
    return in_maps


_PROGRAM = None


def _get_program():
    global _PROGRAM
    if _PROGRAM is None:
        _PROGRAM = _build_program()
    return _PROGRAM


def run_sharded(in_maps, trace=False, **kw):
    from concourse import bass_utils
    nc = _get_program()
    return bass_utils.run_bass_kernel_spmd(
        nc, in_maps, core_ids=list(range(N_CORES)), trace=trace, **kw)


def _assemble(results):
    outf = np.empty((B, HID, D, H, W), np.float32)
    for c in range(N_CORES):
        raw = np.asarray(results[c]["out"])  # [S, HID, D]
        o = raw.reshape(B, HSH, W, HID, D).transpose(0, 3, 4, 1, 2)
        outf[:, :, :, c * HSH:(c + 1) * HSH, :] = o
    return outf


def kernel(x, W, b):
    x = np.asarray(x, np.float32)
    W = np.asarray(W, np.float32)
    b = np.asarray(b, np.float32)
    in_maps = _host_inputs(x, W, b)
    res = run_sharded(in_maps)
    return _assemble(res.results)
